# revision 19
# baseline (speedup 1.0000x reference)
"""Trainium2 Bass kernel for nn_Attention_layer (GNN message passing attention).

Math (see harness reference):
  x_Q = [input_x, pe_Q]  (N, 1024);  x_K = [input_x, pe_K]
  Q = x_Q @ WQ[h] + qb;  K = x_K @ WK[h] + kb;  V = input_x @ WV[h] + vb
  attn = softmax(Q K^T / 16, axis=k);  out = concat_h(attn @ V) @ lin_w.T + lin_b

Distribution (v5): 8 NeuronCores = 4 q-blocks x 2 head-groups.  Core i
handles q-rows [1024*(i//2), +1024) for heads [4*(i%2), +4).  Each core
returns the PARTIAL final-linear product over its 128 local hidden dims;
the host sums core pairs and adds lin_b (no on-device collectives).  This
halves the per-core K/V projection work vs. pure q-sharding (K/V outputs
are 128 dims instead of 256) -- the projections were ~35us of PE wall.

Everything is computed in the transposed domain (scores^T with k-nodes on
partitions) so no on-device transposes are needed.

Engine budget per 4-head group (q-chunk 512 x k-chunk 128):
  - 4 scores MMs (row-tiled quadrants) -> st012 (3-bank tile) + st3.
  - exp split alternates by group parity: even kc ACT exps heads 0,1 in one
    [128,1024] ACTIVATE + DVE heads 2,3; odd kc ACT takes heads 0-2 in one
    [128,1536] ACTIVATE + DVE head 3.  Merged ACT calls amortize the
    ~352-cycle ACTIVATE startup; the alternation balances ACT/DVE load.
  - PV and Z (ones-row) matmuls accumulate IN PSUM across all 32 k-chunks.
  - V_bias is folded into lin_b on the host (attn rows sum to 1).
  - projections interleave into the group stream via a 2-bank PSUM work
    ring; input DMA split across both hwdge queues.

Measured laws (this hw):
  - PE HAM-throttles between K=8/8 (~379ns per 512-col MM) and K=4/8
    (~600ns) run-to-run; compare runs via the median scores-MM duration.
  - ACT ACTIVATE ~ (cols+352)/1.2GHz; DVE [128,512] PSUM op ~683ns (1x,
    fp32 PSUM source caps DVE modes).
  - do NOT interleave MMs of two accumulation chains at the same tile
    position (LDWEIGHTS clobber corrupts silently).
  - fp8 (e4m3) projections fail the 2e-2 gate (4-5e-2 end to end).
"""

import os
import sys
import numpy as np
import ml_dtypes

for _p in ("/opt/trn_rl_repo", "/root/.axon_site/_ro/trn_rl_repo"):
    if os.path.isdir(_p) and _p not in sys.path:
        sys.path.insert(0, _p)

N = 4096
IND = 256          # input_x dim
QKD = 1024         # concat dim for Q/K projections
H = 8              # heads
HD = 32            # head dim
HID = 256          # H * HD
HIDL = 128         # local hidden dims per core (4 heads x 32)
NCORES = 8
NQ2 = 1024         # q rows per core (4 q-blocks x 2 head-groups)
NQ = 512           # q chunk processed per phase
SCALE = 1.0 / 16.0  # 1/sqrt(HID)

# Schraudolph exp constants: one VectorE tensor_scalar writing int16
# produces the BF16 bit pattern of exp(s/16).  C=366000 minimizes max rel
# err (~3.3%).
A_EXP = float((2.0 ** 23) * np.log2(np.e) / 16.0 / 65536.0)
B_EXP = float((127.0 * 2.0 ** 23 - 366000.0) / 65536.0)

_CACHE = {}


def _build_nc():
    from contextlib import ExitStack
    import concourse.bacc as bacc
    import concourse.tile as tile
    import concourse.mybir as mybir
    from concourse.bass import ds, ts

    f32 = mybir.dt.float32
    i16 = mybir.dt.int16
    bf16 = mybir.dt.bfloat16
    Exp = mybir.ActivationFunctionType.Exp
    mult = mybir.AluOpType.mult
    add = mybir.AluOpType.add

    nc = bacc.Bacc("TRN2", target_bir_lowering=False, debug=False,
                   num_devices=NCORES)

    # ---- DRAM I/O (per-core shards prepared on host) ----
    xkT = nc.dram_tensor("xkT", [QKD, N], bf16, kind="ExternalInput")    # [x;peK]^T
    xqT = nc.dram_tensor("xqT", [QKD, NQ2], bf16, kind="ExternalInput")  # q-block
    wq = nc.dram_tensor("wq", [QKD, HIDL], bf16, kind="ExternalInput")   # local heads
    wk = nc.dram_tensor("wk", [QKD, HIDL], bf16, kind="ExternalInput")
    wv = nc.dram_tensor("wv", [IND, HIDL], bf16, kind="ExternalInput")
    lwT = nc.dram_tensor("lwT", [HIDL, HID], bf16, kind="ExternalInput")  # local rows
    bias2 = nc.dram_tensor("bias2", [128, 2], f32, kind="ExternalInput")  # Q,K bias
    out = nc.dram_tensor("out", [HID, NQ2], f32, kind="ExternalOutput")   # partial^T

    # Z-row gather constants (local heads j at partitions 32j).
    selz_np = np.zeros((128, 4), dtype=np.float32)
    for r in range(4):
        selz_np[32 * r, r] = 1.0
    selz_dram = nc.inline_tensor(np.ascontiguousarray(selz_np), name="selz_const")
    bsel_np = np.zeros((4, 128), dtype=np.float32)
    for j in range(4):
        bsel_np[j, 32 * j:32 * j + 32] = 1.0
    bsel_dram = nc.inline_tensor(bsel_np, name="bsel_const")
    ones_np = np.ones((128, 1), dtype=ml_dtypes.bfloat16)
    ones_dram = nc.inline_tensor(ones_np, name="ones_const")

    with tile.TileContext(nc) as tc, ExitStack() as ctx:
        consts = ctx.enter_context(tc.tile_pool(name="consts", bufs=1))
        big = ctx.enter_context(tc.tile_pool(name="big", bufs=1))
        ptp = ctx.enter_context(tc.tile_pool(name="ptp", bufs=3))
        stp = ctx.enter_context(tc.tile_pool(name="stp", bufs=1, space="PSUM"))

        # ---- SBUF tiles ----
        xkt = big.tile([128, 8, N], bf16, tag="xkt")        # x_K^T (8 c-chunks)
        xqt = big.tile([128, 8, NQ2], bf16, tag="xqt")      # x_Q^T block
        wqt = consts.tile([128, 8, HIDL], bf16, tag="wqt")
        wkt = consts.tile([128, 8, HIDL], bf16, tag="wkt")
        wvt = consts.tile([128, 2, HIDL], bf16, tag="wvt")
        lwt = consts.tile([128, HID], bf16, tag="lwt")      # [local-dim, out]
        bt = consts.tile([128, 2], f32, tag="bt")
        selz = consts.tile([128, 4], f32, tag="selz")
        bsel = consts.tile([4, 128], f32, tag="bsel")
        ones = consts.tile([128, 1], bf16, tag="ones")

        kt = big.tile([128, N], bf16, tag="kt")             # K^T (local heads)
        qt = big.tile([128, 2, NQ], bf16, tag="qt")         # Q^T per q-phase
        vt = big.tile([128, 32, HIDL], bf16, tag="vt")      # V node-major
        pvs = big.tile([128, 2, NQ], f32, tag="pvs")        # PV accum drains
        zsb = big.tile([128, 2, NQ], f32, tag="zsb")        # Z accum drains
        zrm = big.tile([4, 2, NQ], f32, tag="zrm")          # 1/Z per head
        attn = big.tile([128, 2, NQ], bf16, tag="attn")     # normalized attn_x^T
        outsb = big.tile([128, 2, 2, NQ], f32, tag="outsb")  # [p, mo, qb, q]

        # ---- const / weight DMAs, ordered by first consumer ----
        # group 0 gates on: wq+xqt (q-proj), wk+xkt-c0 (k-proj); wvt+ones
        # next (first pvz); bt at the first drains; lwt/selz/bsel only at
        # normalization/epilogue -> queue them last.
        xkT_r = xkT.rearrange("(c p) (n q) -> n p c q", p=128, q=512)
        xqT_r = xqT.rearrange("(c p) q -> p c q", p=128)
        nc.sync.dma_start(wqt[:], wq.rearrange("(c p) o -> p c o", p=128))
        nc.sync.dma_start(wkt[:], wk.rearrange("(c p) o -> p c o", p=128))
        nc.scalar.dma_start(xqt[:, :4], xqT_r[:, :4])
        nc.scalar.dma_start(xqt[:, 4:], xqT_r[:, 4:])
        nc.scalar.dma_start(xkt[:, :, ds(0, 128)], xkT_r[0][:, :, ds(0, 128)])
        nc.sync.dma_start(xkt[:, :, ds(128, 384)], xkT_r[0][:, :, ds(128, 384)])
        nc.sync.dma_start(wvt[:], wv.rearrange("(c p) o -> p c o", p=128))
        nc.sync.dma_start(ones[:], ones_dram[:])
        nc.scalar.dma_start(bt[:], bias2[:])
        # dual-queue: scalar's HWDGE queue carries half of each node chunk
        # concurrently with sync's half
        for n in range(1, 8):
            nc.scalar.dma_start(xkt[:, :4, ts(n, 512)], xkT_r[n][:, :4])
            nc.sync.dma_start(xkt[:, 4:, ts(n, 512)], xkT_r[n][:, 4:])
        nc.sync.dma_start(lwt[:], lwT[:])
        nc.sync.dma_start(selz[:], selz_dram[:])
        nc.sync.dma_start(bsel[:], bsel_dram[:])

        # preload the ACT exp table set while DMAs land
        actwarm = consts.tile([8, 16], f32, tag="actwarm")
        nc.vector.memset(actwarm[:], 0.0)
        nc.scalar.activation(actwarm[:], actwarm[:], Exp)

        # ---- projection units (PSUM work ring, 2 banks) ----
        # Two independent half-units interleave their MMs so the PE streams
        # 2-wide (adjacent MMs hit different banks).
        def qk_proj_pair(specs):
            # specs: list of 2 (kind, n) where kind in 'q','k'; for 'q' n is
            # the q-phase, for 'k' the 512-node block.
            tiles = []
            for kind, n in specs:
                ps = stp.tile([128, NQ], f32, tag="work", bufs=2,
                              name=f"{kind}p{n}")
                tiles.append(ps)
            for (kind, n), ps in zip(specs, tiles):
                for c in range(8):
                    if kind == 'q':
                        nc.tensor.matmul(ps[:, :NQ], wqt[:, c, :],
                                         xqt[:, c, ds(512 * n, 512)],
                                         start=(c == 0), stop=(c == 7))
                    else:
                        nc.tensor.matmul(ps[:, :NQ], wkt[:, c, :],
                                         xkt[:, c, ts(n, 512)],
                                         start=(c == 0), stop=(c == 7))
            for (kind, n), ps in zip(specs, tiles):
                if kind == 'q':
                    nc.vector.tensor_scalar_add(qt[:, n, :], ps[:, :NQ],
                                                bt[:, 0:1])
                else:
                    nc.vector.tensor_scalar_add(kt[:, ts(n, 512)], ps[:, :NQ],
                                                bt[:, 1:2])

        def v_proj_unit(g):
            # covers node chunks 2g, 2g+1 -> vt[:, 2g:2g+2, :] (2x128 cols).
            ps = stp.tile([128, NQ], f32, tag="work", bufs=2, name=f"vp{g}")
            for kc in (2 * g, 2 * g + 1):
                off = HIDL * (kc - 2 * g)
                for c in range(2):
                    nc.tensor.matmul(ps[:, ds(off, HIDL)],
                                     xkt[:, c, ds(128 * kc, 128)],
                                     wvt[:, c, :], start=(c == 0), stop=(c == 1))
            nc.vector.tensor_copy(out=vt[:, 2 * g:2 * g + 2, :],
                                  in_=ps[:, ds(0, 2 * HIDL)].rearrange(
                                      "p (g o) -> p g o", o=HIDL))

        # ---- prologue: minimum for attention group (kc=0, qb=0) ----
        qk_proj_pair([('q', 0), ('k', 0)])
        v_proj_unit(0)

        # remaining proj work, scheduled into the group stream.
        # group index g = 32*qb + kc (qb outer).  k_proj(n) must complete
        # before group 4n; v_proj(g') before group 2g'; q(1) before group 32.
        pre_work = {}

        def sched(slot, fn):
            pre_work.setdefault(max(0, slot), []).append(fn)

        sched(0, lambda: qk_proj_pair([('k', 1), ('q', 1)]))
        sched(3, lambda: qk_proj_pair([('k', 2), ('k', 3)]))
        sched(9, lambda: qk_proj_pair([('k', 4), ('k', 5)]))
        sched(17, lambda: qk_proj_pair([('k', 6), ('k', 7)]))
        for g in range(1, 16):
            sched(2 * g - 2, lambda g=g: v_proj_unit(g))

        # ---- main attention loop: qb outer, 32 k-chunks inner ----
        def pvz_unit(pt, kc, pvacc, zacc):
            for j in range(4):
                nc.tensor.matmul(
                    pvacc[ds(32 * j, 32), :],
                    vt[:, kc, ds(32 * j, 32)],
                    pt[:, ts(j, NQ)],
                    start=(kc == 0), stop=(kc == 31),
                    tile_position=(0, 32 * j))
            for j in range(4):
                nc.tensor.matmul(
                    zacc[ds(32 * j, 1), :],
                    ones[:],
                    pt[:, ts(j, NQ)],
                    start=(kc == 0), stop=(kc == 31),
                    tile_position=(0, 32 * j))

        prev = None
        for qb in range(2):
            pvacc = stp.tile([128, NQ], f32, tag="pv", bufs=1, name=f"pvacc{qb}")
            zacc = stp.tile([128, NQ], f32, tag="z", bufs=1, name=f"zacc{qb}")
            # rows of zacc outside {0,32,64,96} are never written by the PE
            # but flow into the selz gather (x0.0) - keep them finite.
            nc.vector.memset(zacc[:], 0.0)
            for kc in range(32):
                g = 32 * qb + kc
                # heads 0-2 share a 3-bank tile st012; head 3 gets st3.
                st012 = stp.tile([128, 3 * NQ], f32, tag="st012", bufs=1,
                                 name="st012")
                st3 = stp.tile([128, NQ], f32, tag="st3", bufs=1, name="st3")
                sth = [st012[:, ds(0, NQ)], st012[:, ds(NQ, NQ)],
                       st012[:, ds(2 * NQ, NQ)], st3]
                # issue order: the bank freed earliest by the previous
                # group's exp goes first (prev odd: st3 freed first).
                jorder = [3, 0, 1, 2] if (kc % 2 == 0 and kc > 0) else \
                         [0, 1, 2, 3]
                for j in jorder:
                    nc.tensor.matmul(
                        sth[j][:, :NQ],
                        kt[ds(32 * j, 32), ds(128 * kc, 128)],
                        qt[ds(32 * j, 32), qb, :],
                        start=True, stop=True,
                        tile_position=(32 * j, 0))
                pt = ptp.tile([128, 4 * NQ], bf16, tag="pt", name="pt")
                pti = pt.bitcast(i16)
                if kc % 2 == 0:
                    nc.scalar.activation(pt[:, ds(0, 2 * NQ)],
                                         st012[:, ds(0, 2 * NQ)], Exp,
                                         scale=SCALE)
                    nc.vector.tensor_scalar(pti[:, ds(2 * NQ, NQ)],
                                            st012[:, ds(2 * NQ, NQ)],
                                            A_EXP, B_EXP, mult, add)
                    nc.vector.tensor_scalar(pti[:, ds(3 * NQ, NQ)],
                                            st3[:, :NQ],
                                            A_EXP, B_EXP, mult, add)
                else:
                    nc.scalar.activation(pt[:, ds(0, 3 * NQ)],
                                         st012[:, :], Exp, scale=SCALE)
                    nc.vector.tensor_scalar(pti[:, ds(3 * NQ, NQ)],
                                            st3[:, :NQ],
                                            A_EXP, B_EXP, mult, add)
                for fn in pre_work.get(g, []):
                    fn()
                if prev is not None:
                    pvz_unit(*prev)
                prev = (pt, kc, pvacc, zacc)
            pvz_unit(*prev)
            prev = None
            # drain this qb's accumulators; normalization chain (qb0's hides
            # inside the qb1 phase; only qb1's is an exposed tail).
            nc.vector.tensor_copy(out=pvs[:, qb, :], in_=pvacc[:])
            # zsb drain on ScalarE: runs concurrently with the pvs copy.
            nc.scalar.copy(zsb[:, qb, :], zacc[:])
            zqm = stp.tile([128, NQ], f32, tag="work", bufs=2, name=f"zq{qb}")
            nc.tensor.matmul(zqm[:4, :NQ], selz[:], zsb[:, qb, :],
                             start=True, stop=True)
            nc.vector.reciprocal_approx_fast(zrm[:, qb, :], zqm[:4, :NQ])
            psb = stp.tile([128, NQ], f32, tag="work", bufs=2, name=f"psb{qb}")
            nc.tensor.matmul(psb[:, :NQ], bsel[:], zrm[:4, qb, :],
                             start=True, stop=True)
            # V_bias is folded into lin_b on the host (attn rows sum to 1).
            nc.vector.tensor_tensor(attn[:, qb, :], pvs[:, qb, :], psb[:, :NQ],
                                    mult)
            # partial final linear for this q-phase (contract = 128 local
            # dims, single chunk); qb0's hides inside the qb1 phase.
            out_r = out.rearrange("(m p) q -> p m q", p=128)
            for mo in range(2):
                lin = stp.tile([128, NQ], f32, tag="work", bufs=2,
                               name=f"lin{qb}_{mo}")
                nc.tensor.matmul(lin[:, :NQ], lwt[:, ts(mo, 128)],
                                 attn[:, qb, :], start=True, stop=True)
                nc.scalar.copy(outsb[:, mo, qb, :], lin[:, :NQ])
                nc.sync.dma_start(out_r[:, mo, ds(512 * qb, 512)],
                                  outsb[:, mo, qb, :])

    nc.compile()
    return nc


def _get_nc():
    if "nc" not in _CACHE:
        _CACHE["nc"] = _build_nc()
    return _CACHE["nc"]


def _prep_in_maps(input_x, pe_Q, pe_K, WQ, WK, WV, Q_bias, K_bias, V_bias,
                  lin_w, lin_b):
    bf = ml_dtypes.bfloat16
    x_kT = np.ascontiguousarray(
        np.concatenate([input_x, pe_K], axis=1).T.astype(bf))       # [1024, 4096]
    x_q = np.concatenate([input_x, pe_Q], axis=1)                   # [4096, 1024]
    wq_h = WQ.transpose(1, 0, 2).reshape(QKD, HID)                  # [d,(h,hd)]
    wk_h = WK.transpose(1, 0, 2).reshape(QKD, HID)
    wv_h = WV.transpose(1, 0, 2).reshape(IND, HID)
    lwTn = lin_w.T                                                  # [in, out]
    qb_full = Q_bias.reshape(HID)
    kb_full = K_bias.reshape(HID)
    in_maps = []
    for i in range(NCORES):
        blk, hg = i // 2, i % 2
        xqT_i = np.ascontiguousarray(
            x_q[blk * NQ2:(blk + 1) * NQ2].T.astype(bf))            # [1024, 1024]
        sl = slice(HIDL * hg, HIDL * (hg + 1))
        bias2 = np.zeros((128, 2), np.float32)
        bias2[:, 0] = qb_full[sl]
        bias2[:, 1] = kb_full[sl]
        in_maps.append({
            "xkT": x_kT, "xqT": xqT_i,
            "wq": np.ascontiguousarray(wq_h[:, sl].astype(bf)),
            "wk": np.ascontiguousarray(wk_h[:, sl].astype(bf)),
            "wv": np.ascontiguousarray(wv_h[:, sl].astype(bf)),
            "lwT": np.ascontiguousarray(lwTn[sl, :].astype(bf)),
            "bias2": bias2,
        })
    return in_maps


def _ensure_ntff_hook():
    """The agent image's antenv lacks axon_hooks; synthesize it from the
    boot script's ctypes NTFF implementation so trace=True works."""
    import types
    try:
        from antenv.axon_hooks import get_axon_ntff_profile_hook  # noqa: F401
        return
    except ImportError:
        pass
    sys.path.insert(0, "/root/.axon_site/trn_agent_boot")
    import trn_boot
    hook = trn_boot._ntff_profile_via_ctypes(
        os.environ.get("PJRT_LIBRARY_PATH", "/opt/axon/libaxon_pjrt.so"))
    mod = types.ModuleType("antenv.axon_hooks")
    mod._hook = hook
    mod.get_axon_ntff_profile_hook = lambda: mod._hook
    mod.set_axon_ntff_profile_hook = lambda h: setattr(mod, "_hook", h)
    sys.modules["antenv.axon_hooks"] = mod


def _run(in_maps, trace=False):
    from concourse.bass_utils import run_bass_kernel_spmd
    if trace:
        _ensure_ntff_hook()
    nc = _get_nc()
    res = run_bass_kernel_spmd(nc, in_maps, core_ids=list(range(NCORES)),
                               trace=trace)
    return res


def _finish(res, V_bias, lin_w, lin_b):
    # host epilogue: sum the two head-group partials per q-block; V_bias
    # (constant through the softmax) and lin_b fold into one bias vector.
    lin_b_eff = (lin_b.reshape(HID) +
                 V_bias.reshape(HID) @ lin_w.T).astype(np.float32)
    out_full = np.empty((N, HID), np.float32)
    for blk in range(4):
        part = res.results[2 * blk]["out"] + res.results[2 * blk + 1]["out"]
        out_full[blk * NQ2:(blk + 1) * NQ2] = part.T + lin_b_eff
    return out_full


def kernel(input_x, pe_Q, pe_K, A, WQ, WK, WV, Q_bias, K_bias, V_bias,
           lin_w, lin_b):
    args = [np.asarray(x, np.float32) for x in
            (input_x, pe_Q, pe_K, WQ, WK, WV, Q_bias, K_bias, V_bias,
             lin_w, lin_b)]
    in_maps = _prep_in_maps(*args)
    res = _run(in_maps)
    return _finish(res, args[8], args[9], args[10])


def hw_exec_ns(input_x, pe_Q, pe_K, A, WQ, WK, WV, Q_bias, K_bias, V_bias,
               lin_w, lin_b):
    """Run once with NTFF tracing; returns (exec_time_ns, results)."""
    args = [np.asarray(x, np.float32) for x in
            (input_x, pe_Q, pe_K, WQ, WK, WV, Q_bias, K_bias, V_bias,
             lin_w, lin_b)]
    in_maps = _prep_in_maps(*args)
    res = _run(in_maps, trace=True)
    return res.exec_time_ns, res


# revision 23
# speedup vs baseline: 1.0121x; 1.0121x over previous
"""Trainium2 Bass kernel for nn_Attention_layer (GNN message passing attention).

Math (see harness reference):
  x_Q = [input_x, pe_Q]  (N, 1024);  x_K = [input_x, pe_K]
  Q = x_Q @ WQ[h] + qb;  K = x_K @ WK[h] + kb;  V = input_x @ WV[h] + vb
  attn = softmax(Q K^T / 16, axis=k);  out = concat_h(attn @ V) @ lin_w.T + lin_b

Distribution (v5): 8 NeuronCores = 4 q-blocks x 2 head-groups.  Core i
handles q-rows [1024*(i//2), +1024) for heads [4*(i%2), +4).  Each core
returns the PARTIAL final-linear product over its 128 local hidden dims;
the host sums core pairs and adds lin_b (no on-device collectives).  This
halves the per-core K/V projection work vs. pure q-sharding (K/V outputs
are 128 dims instead of 256) -- the projections were ~35us of PE wall.

Everything is computed in the transposed domain (scores^T with k-nodes on
partitions) so no on-device transposes are needed.

Engine budget per 4-head group (q-chunk 512 x k-chunk 128):
  - 4 scores MMs (row-tiled quadrants) -> st012 (3-bank tile) + st3.
  - exp split alternates by group parity: even kc ACT exps heads 0,1 in one
    [128,1024] ACTIVATE + DVE heads 2,3; odd kc ACT takes heads 0-2 in one
    [128,1536] ACTIVATE + DVE head 3.  Merged ACT calls amortize the
    ~352-cycle ACTIVATE startup; the alternation balances ACT/DVE load.
  - PV and Z (ones-row) matmuls accumulate IN PSUM across all 32 k-chunks.
  - V_bias is folded into lin_b on the host (attn rows sum to 1).
  - projections interleave into the group stream via a 2-bank PSUM work
    ring; input DMA split across both hwdge queues.

Measured laws (this hw):
  - PE HAM-throttles between K=8/8 (~379ns per 512-col MM) and K=4/8
    (~600ns) run-to-run; compare runs via the median scores-MM duration.
  - ACT ACTIVATE ~ (cols+352)/1.2GHz; DVE [128,512] PSUM op ~683ns (1x,
    fp32 PSUM source caps DVE modes).
  - do NOT interleave MMs of two accumulation chains at the same tile
    position (LDWEIGHTS clobber corrupts silently).
  - fp8 (e4m3) projections fail the 2e-2 gate (4-5e-2 end to end).
"""

import os
import sys
import numpy as np
import ml_dtypes

for _p in ("/opt/trn_rl_repo", "/root/.axon_site/_ro/trn_rl_repo"):
    if os.path.isdir(_p) and _p not in sys.path:
        sys.path.insert(0, _p)

N = 4096
IND = 256          # input_x dim
QKD = 1024         # concat dim for Q/K projections
H = 8              # heads
HD = 32            # head dim
HID = 256          # H * HD
HIDL = 128         # local hidden dims per core (4 heads x 32)
NCORES = 8
NQ2 = 1024         # q rows per core (4 q-blocks x 2 head-groups)
NQ = 512           # q chunk processed per phase
SCALE = 1.0 / 16.0  # 1/sqrt(HID)

# Schraudolph exp constants: one VectorE tensor_scalar writing int16
# produces the BF16 bit pattern of exp(s/16).  C=366000 minimizes max rel
# err (~3.3%).
A_EXP = float((2.0 ** 23) * np.log2(np.e) / 16.0 / 65536.0)
B_EXP = float((127.0 * 2.0 ** 23 - 366000.0) / 65536.0)

_CACHE = {}


def _build_nc():
    from contextlib import ExitStack
    import concourse.bacc as bacc
    import concourse.tile as tile
    import concourse.mybir as mybir
    from concourse.bass import ds, ts

    f32 = mybir.dt.float32
    i16 = mybir.dt.int16
    bf16 = mybir.dt.bfloat16
    Exp = mybir.ActivationFunctionType.Exp
    mult = mybir.AluOpType.mult
    add = mybir.AluOpType.add

    nc = bacc.Bacc("TRN2", target_bir_lowering=False, debug=False,
                   num_devices=NCORES)

    # ---- DRAM I/O (per-core shards prepared on host) ----
    xkT = nc.dram_tensor("xkT", [QKD, N], bf16, kind="ExternalInput")    # [x;peK]^T
    xqT = nc.dram_tensor("xqT", [QKD, NQ2], bf16, kind="ExternalInput")  # q-block
    wq = nc.dram_tensor("wq", [QKD, HIDL], bf16, kind="ExternalInput")   # local heads
    wk = nc.dram_tensor("wk", [QKD, HIDL], bf16, kind="ExternalInput")
    wv = nc.dram_tensor("wv", [IND, HIDL], bf16, kind="ExternalInput")
    lwT = nc.dram_tensor("lwT", [HIDL, HID], bf16, kind="ExternalInput")  # local rows
    bias2 = nc.dram_tensor("bias2", [128, 2], f32, kind="ExternalInput")  # Q,K bias
    out = nc.dram_tensor("out", [HID, NQ2], f32, kind="ExternalOutput")   # partial^T

    # Z-row gather constants (local heads j at partitions 32j).
    selz_np = np.zeros((128, 4), dtype=np.float32)
    for r in range(4):
        selz_np[32 * r, r] = 1.0
    selz_dram = nc.inline_tensor(np.ascontiguousarray(selz_np), name="selz_const")
    bsel_np = np.zeros((4, 128), dtype=np.float32)
    for j in range(4):
        bsel_np[j, 32 * j:32 * j + 32] = 1.0
    bsel_dram = nc.inline_tensor(bsel_np, name="bsel_const")
    ones_np = np.ones((128, 1), dtype=ml_dtypes.bfloat16)
    ones_dram = nc.inline_tensor(ones_np, name="ones_const")

    with tile.TileContext(nc) as tc, ExitStack() as ctx:
        consts = ctx.enter_context(tc.tile_pool(name="consts", bufs=1))
        big = ctx.enter_context(tc.tile_pool(name="big", bufs=1))
        ptp = ctx.enter_context(tc.tile_pool(name="ptp", bufs=3))
        stp = ctx.enter_context(tc.tile_pool(name="stp", bufs=1, space="PSUM"))

        # ---- SBUF tiles ----
        xkt = big.tile([128, 8, N], bf16, tag="xkt")        # x_K^T (8 c-chunks)
        xqt = big.tile([128, 8, NQ2], bf16, tag="xqt")      # x_Q^T block
        wqt = consts.tile([128, 8, HIDL], bf16, tag="wqt")
        wkt = consts.tile([128, 8, HIDL], bf16, tag="wkt")
        wvt = consts.tile([128, 2, HIDL], bf16, tag="wvt")
        lwt = consts.tile([128, HID], bf16, tag="lwt")      # [local-dim, out]
        bt = consts.tile([128, 2], f32, tag="bt")
        selz = consts.tile([128, 4], f32, tag="selz")
        bsel = consts.tile([4, 128], f32, tag="bsel")
        ones = consts.tile([128, 1], bf16, tag="ones")

        kt = big.tile([128, N], bf16, tag="kt")             # K^T (local heads)
        qt = big.tile([128, 2, NQ], bf16, tag="qt")         # Q^T per q-phase
        vt = big.tile([128, 32, HIDL], bf16, tag="vt")      # V node-major
        pvs = big.tile([128, 2, NQ], f32, tag="pvs")        # PV accum drains
        zsb = big.tile([128, 2, NQ], f32, tag="zsb")        # Z accum drains
        zrm = big.tile([4, 2, NQ], f32, tag="zrm")          # 1/Z per head
        attn = big.tile([128, 2, NQ], bf16, tag="attn")     # normalized attn_x^T
        outsb = big.tile([128, 2, 2, NQ], f32, tag="outsb")  # [p, mo, qb, q]

        # ---- const / weight DMAs, ordered by first consumer ----
        # group 0 gates on: wq+xqt (q-proj), wk+xkt-c0 (k-proj); wvt+ones
        # next (first pvz); bt at the first drains; lwt/selz/bsel only at
        # normalization/epilogue -> queue them last.
        xkT_r = xkT.rearrange("(c p) (n q) -> n p c q", p=128, q=512)
        xqT_r = xqT.rearrange("(c p) q -> p c q", p=128)
        nc.sync.dma_start(wqt[:], wq.rearrange("(c p) o -> p c o", p=128))
        nc.sync.dma_start(wkt[:], wk.rearrange("(c p) o -> p c o", p=128))
        # xqt split by q-phase: q-proj(0) only needs cols 0:512 (all 8 c),
        # so phase-0's half lands first; phase-1's half arrives mid-stream
        # (consumed around group 20).
        nc.scalar.dma_start(xqt[:, :, ds(0, 512)], xqT_r[:, :, ds(0, 512)])
        nc.scalar.dma_start(xkt[:, :, ds(0, 128)], xkT_r[0][:, :, ds(0, 128)])
        nc.scalar.dma_start(bt[:], bias2[:])
        nc.sync.dma_start(xkt[:, :, ds(128, 384)], xkT_r[0][:, :, ds(128, 384)])
        nc.sync.dma_start(wvt[:], wv.rearrange("(c p) o -> p c o", p=128))
        nc.sync.dma_start(ones[:], ones_dram[:])
        # dual-queue: scalar's HWDGE queue carries half of each node chunk
        # concurrently with sync's half
        for n in range(1, 6):
            nc.scalar.dma_start(xkt[:, :4, ts(n, 512)], xkT_r[n][:, :4])
            nc.sync.dma_start(xkt[:, 4:, ts(n, 512)], xkT_r[n][:, 4:])
        nc.scalar.dma_start(xqt[:, :, ds(512, 512)], xqT_r[:, :, ds(512, 512)])
        for n in range(6, 8):
            nc.scalar.dma_start(xkt[:, :4, ts(n, 512)], xkT_r[n][:, :4])
            nc.sync.dma_start(xkt[:, 4:, ts(n, 512)], xkT_r[n][:, 4:])
        nc.sync.dma_start(lwt[:], lwT[:])
        nc.sync.dma_start(selz[:], selz_dram[:])
        nc.sync.dma_start(bsel[:], bsel_dram[:])

        # preload the ACT exp table set while DMAs land
        actwarm = consts.tile([8, 16], f32, tag="actwarm")
        nc.vector.memset(actwarm[:], 0.0)
        nc.scalar.activation(actwarm[:], actwarm[:], Exp)

        # ---- projection units (PSUM work ring, 2 banks) ----
        # Two independent half-units interleave their MMs so the PE streams
        # 2-wide (adjacent MMs hit different banks).
        def qk_proj_pair(specs):
            # specs: list of 2 (kind, n) where kind in 'q','k'; for 'q' n is
            # the q-phase, for 'k' the 512-node block.
            tiles = []
            for kind, n in specs:
                ps = stp.tile([128, NQ], f32, tag="work", bufs=2,
                              name=f"{kind}p{n}")
                tiles.append(ps)
            for (kind, n), ps in zip(specs, tiles):
                for c in range(8):
                    if kind == 'q':
                        nc.tensor.matmul(ps[:, :NQ], wqt[:, c, :],
                                         xqt[:, c, ds(512 * n, 512)],
                                         start=(c == 0), stop=(c == 7))
                    else:
                        nc.tensor.matmul(ps[:, :NQ], wkt[:, c, :],
                                         xkt[:, c, ts(n, 512)],
                                         start=(c == 0), stop=(c == 7))
            for (kind, n), ps in zip(specs, tiles):
                if kind == 'q':
                    nc.vector.tensor_scalar_add(qt[:, n, :], ps[:, :NQ],
                                                bt[:, 0:1])
                else:
                    nc.vector.tensor_scalar_add(kt[:, ts(n, 512)], ps[:, :NQ],
                                                bt[:, 1:2])

        def v_proj_unit(g):
            # covers node chunks 2g, 2g+1 -> vt[:, 2g:2g+2, :] (2x128 cols).
            # Drains alternate VectorE/ScalarE to balance the exp queues.
            ps = stp.tile([128, NQ], f32, tag="work", bufs=2, name=f"vp{g}")
            for kc in (2 * g, 2 * g + 1):
                off = HIDL * (kc - 2 * g)
                for c in range(2):
                    nc.tensor.matmul(ps[:, ds(off, HIDL)],
                                     xkt[:, c, ds(128 * kc, 128)],
                                     wvt[:, c, :], start=(c == 0), stop=(c == 1))
            src = ps[:, ds(0, 2 * HIDL)].rearrange("p (g o) -> p g o", o=HIDL)
            if g % 2 == 0:
                nc.vector.tensor_copy(out=vt[:, 2 * g:2 * g + 2, :], in_=src)
            else:
                nc.scalar.copy(vt[:, 2 * g:2 * g + 2, :], src)

        # ---- prologue: minimum for attention group (kc=0, qb=0) ----
        qk_proj_pair([('q', 0), ('k', 0)])
        v_proj_unit(0)

        # remaining proj work, scheduled into the group stream.
        # group index g = 32*qb + kc (qb outer).  k_proj(n) must complete
        # before group 4n; v_proj(g') before group 2g'; q(1) before group 32.
        pre_work = {}

        def sched(slot, fn):
            pre_work.setdefault(max(0, slot), []).append(fn)

        sched(0, lambda: qk_proj_pair([('k', 1)]))
        sched(3, lambda: qk_proj_pair([('k', 2), ('k', 3)]))
        sched(9, lambda: qk_proj_pair([('k', 4), ('k', 5)]))
        sched(17, lambda: qk_proj_pair([('k', 6), ('k', 7)]))
        sched(20, lambda: qk_proj_pair([('q', 1)]))  # xqt phase-1 lands late
        for g in range(1, 16):
            sched(2 * g - 2, lambda g=g: v_proj_unit(g))

        # ---- main attention loop: qb outer, 32 k-chunks inner ----
        def pvz_unit(pt, kc, pvacc, zacc):
            for j in range(4):
                nc.tensor.matmul(
                    pvacc[ds(32 * j, 32), :],
                    vt[:, kc, ds(32 * j, 32)],
                    pt[:, ts(j, NQ)],
                    start=(kc == 0), stop=(kc == 31),
                    tile_position=(0, 32 * j))
            for j in range(4):
                nc.tensor.matmul(
                    zacc[ds(32 * j, 1), :],
                    ones[:],
                    pt[:, ts(j, NQ)],
                    start=(kc == 0), stop=(kc == 31),
                    tile_position=(0, 32 * j))

        prev = None
        for qb in range(2):
            pvacc = stp.tile([128, NQ], f32, tag="pv", bufs=1, name=f"pvacc{qb}")
            zacc = stp.tile([128, NQ], f32, tag="z", bufs=1, name=f"zacc{qb}")
            # rows of zacc outside {0,32,64,96} are never written by the PE
            # but flow into the selz gather (x0.0) - keep them finite.
            nc.vector.memset(zacc[:], 0.0)
            for kc in range(32):
                g = 32 * qb + kc
                # heads 0-2 share a 3-bank tile st012; head 3 gets st3.
                st012 = stp.tile([128, 3 * NQ], f32, tag="st012", bufs=1,
                                 name="st012")
                st3 = stp.tile([128, NQ], f32, tag="st3", bufs=1, name="st3")
                sth = [st012[:, ds(0, NQ)], st012[:, ds(NQ, NQ)],
                       st012[:, ds(2 * NQ, NQ)], st3]
                # ACT exps 3 heads every 4th group, else 2 -- ~2.25 heads
                # average balances ACT (merged-call rate) against DVE
                # (per-op overhead + drain work).
                act3 = (kc % 4 == 1)
                prev3 = (kc % 4 == 2)
                # issue order: the bank freed earliest by the previous
                # group's exp goes first (prev 3/1 group: st3 freed first).
                jorder = [3, 0, 1, 2] if prev3 else [0, 1, 2, 3]
                for j in jorder:
                    nc.tensor.matmul(
                        sth[j][:, :NQ],
                        kt[ds(32 * j, 32), ds(128 * kc, 128)],
                        qt[ds(32 * j, 32), qb, :],
                        start=True, stop=True,
                        tile_position=(32 * j, 0))
                pt = ptp.tile([128, 4 * NQ], bf16, tag="pt", name="pt")
                pti = pt.bitcast(i16)
                if act3:
                    nc.scalar.activation(pt[:, ds(0, 3 * NQ)],
                                         st012[:, :], Exp, scale=SCALE)
                    nc.vector.tensor_scalar(pti[:, ds(3 * NQ, NQ)],
                                            st3[:, :NQ],
                                            A_EXP, B_EXP, mult, add)
                else:
                    nc.scalar.activation(pt[:, ds(0, 2 * NQ)],
                                         st012[:, ds(0, 2 * NQ)], Exp,
                                         scale=SCALE)
                    nc.vector.tensor_scalar(pti[:, ds(2 * NQ, NQ)],
                                            st012[:, ds(2 * NQ, NQ)],
                                            A_EXP, B_EXP, mult, add)
                    nc.vector.tensor_scalar(pti[:, ds(3 * NQ, NQ)],
                                            st3[:, :NQ],
                                            A_EXP, B_EXP, mult, add)
                for fn in pre_work.get(g, []):
                    fn()
                if prev is not None:
                    pvz_unit(*prev)
                prev = (pt, kc, pvacc, zacc)
            pvz_unit(*prev)
            prev = None
            # drain this qb's accumulators; normalization chain (qb0's hides
            # inside the qb1 phase; only qb1's is an exposed tail).
            nc.vector.tensor_copy(out=pvs[:, qb, :], in_=pvacc[:])
            # zsb drain on ScalarE: runs concurrently with the pvs copy.
            nc.scalar.copy(zsb[:, qb, :], zacc[:])
            zqm = stp.tile([128, NQ], f32, tag="work", bufs=2, name=f"zq{qb}")
            nc.tensor.matmul(zqm[:4, :NQ], selz[:], zsb[:, qb, :],
                             start=True, stop=True)
            nc.vector.reciprocal_approx_fast(zrm[:, qb, :], zqm[:4, :NQ])
            psb = stp.tile([128, NQ], f32, tag="work", bufs=2, name=f"psb{qb}")
            nc.tensor.matmul(psb[:, :NQ], bsel[:], zrm[:4, qb, :],
                             start=True, stop=True)
            # V_bias is folded into lin_b on the host (attn rows sum to 1).
            nc.vector.tensor_tensor(attn[:, qb, :], pvs[:, qb, :], psb[:, :NQ],
                                    mult)
            # partial final linear for this q-phase (contract = 128 local
            # dims, single chunk); qb0's hides inside the qb1 phase.
            out_r = out.rearrange("(m p) q -> p m q", p=128)
            for mo in range(2):
                lin = stp.tile([128, NQ], f32, tag="work", bufs=2,
                               name=f"lin{qb}_{mo}")
                nc.tensor.matmul(lin[:, :NQ], lwt[:, ts(mo, 128)],
                                 attn[:, qb, :], start=True, stop=True)
                nc.scalar.copy(outsb[:, mo, qb, :], lin[:, :NQ])
                nc.sync.dma_start(out_r[:, mo, ds(512 * qb, 512)],
                                  outsb[:, mo, qb, :])

    nc.compile()
    return nc


def _get_nc():
    if "nc" not in _CACHE:
        _CACHE["nc"] = _build_nc()
    return _CACHE["nc"]


def _prep_in_maps(input_x, pe_Q, pe_K, WQ, WK, WV, Q_bias, K_bias, V_bias,
                  lin_w, lin_b):
    bf = ml_dtypes.bfloat16
    x_kT = np.ascontiguousarray(
        np.concatenate([input_x, pe_K], axis=1).T.astype(bf))       # [1024, 4096]
    x_q = np.concatenate([input_x, pe_Q], axis=1)                   # [4096, 1024]
    wq_h = WQ.transpose(1, 0, 2).reshape(QKD, HID)                  # [d,(h,hd)]
    wk_h = WK.transpose(1, 0, 2).reshape(QKD, HID)
    wv_h = WV.transpose(1, 0, 2).reshape(IND, HID)
    lwTn = lin_w.T                                                  # [in, out]
    qb_full = Q_bias.reshape(HID)
    kb_full = K_bias.reshape(HID)
    in_maps = []
    for i in range(NCORES):
        blk, hg = i // 2, i % 2
        xqT_i = np.ascontiguousarray(
            x_q[blk * NQ2:(blk + 1) * NQ2].T.astype(bf))            # [1024, 1024]
        sl = slice(HIDL * hg, HIDL * (hg + 1))
        bias2 = np.zeros((128, 2), np.float32)
        bias2[:, 0] = qb_full[sl]
        bias2[:, 1] = kb_full[sl]
        in_maps.append({
            "xkT": x_kT, "xqT": xqT_i,
            "wq": np.ascontiguousarray(wq_h[:, sl].astype(bf)),
            "wk": np.ascontiguousarray(wk_h[:, sl].astype(bf)),
            "wv": np.ascontiguousarray(wv_h[:, sl].astype(bf)),
            "lwT": np.ascontiguousarray(lwTn[sl, :].astype(bf)),
            "bias2": bias2,
        })
    return in_maps


def _ensure_ntff_hook():
    """The agent image's antenv lacks axon_hooks; synthesize it from the
    boot script's ctypes NTFF implementation so trace=True works."""
    import types
    try:
        from antenv.axon_hooks import get_axon_ntff_profile_hook  # noqa: F401
        return
    except ImportError:
        pass
    sys.path.insert(0, "/root/.axon_site/trn_agent_boot")
    import trn_boot
    hook = trn_boot._ntff_profile_via_ctypes(
        os.environ.get("PJRT_LIBRARY_PATH", "/opt/axon/libaxon_pjrt.so"))
    mod = types.ModuleType("antenv.axon_hooks")
    mod._hook = hook
    mod.get_axon_ntff_profile_hook = lambda: mod._hook
    mod.set_axon_ntff_profile_hook = lambda h: setattr(mod, "_hook", h)
    sys.modules["antenv.axon_hooks"] = mod


def _run(in_maps, trace=False):
    from concourse.bass_utils import run_bass_kernel_spmd
    if trace:
        _ensure_ntff_hook()
    nc = _get_nc()
    res = run_bass_kernel_spmd(nc, in_maps, core_ids=list(range(NCORES)),
                               trace=trace)
    return res


def _finish(res, V_bias, lin_w, lin_b):
    # host epilogue: sum the two head-group partials per q-block; V_bias
    # (constant through the softmax) and lin_b fold into one bias vector.
    lin_b_eff = (lin_b.reshape(HID) +
                 V_bias.reshape(HID) @ lin_w.T).astype(np.float32)
    out_full = np.empty((N, HID), np.float32)
    for blk in range(4):
        part = res.results[2 * blk]["out"] + res.results[2 * blk + 1]["out"]
        out_full[blk * NQ2:(blk + 1) * NQ2] = part.T + lin_b_eff
    return out_full


def kernel(input_x, pe_Q, pe_K, A, WQ, WK, WV, Q_bias, K_bias, V_bias,
           lin_w, lin_b):
    args = [np.asarray(x, np.float32) for x in
            (input_x, pe_Q, pe_K, WQ, WK, WV, Q_bias, K_bias, V_bias,
             lin_w, lin_b)]
    in_maps = _prep_in_maps(*args)
    res = _run(in_maps)
    return _finish(res, args[8], args[9], args[10])


def hw_exec_ns(input_x, pe_Q, pe_K, A, WQ, WK, WV, Q_bias, K_bias, V_bias,
               lin_w, lin_b):
    """Run once with NTFF tracing; returns (exec_time_ns, results)."""
    args = [np.asarray(x, np.float32) for x in
            (input_x, pe_Q, pe_K, WQ, WK, WV, Q_bias, K_bias, V_bias,
             lin_w, lin_b)]
    in_maps = _prep_in_maps(*args)
    res = _run(in_maps, trace=True)
    return res.exec_time_ns, res


# revision 26
# speedup vs baseline: 1.0166x; 1.0044x over previous
"""Trainium2 Bass kernel for nn_Attention_layer (GNN message passing attention).

Math (see harness reference):
  x_Q = [input_x, pe_Q]  (N, 1024);  x_K = [input_x, pe_K]
  Q = x_Q @ WQ[h] + qb;  K = x_K @ WK[h] + kb;  V = input_x @ WV[h] + vb
  attn = softmax(Q K^T / 16, axis=k);  out = concat_h(attn @ V) @ lin_w.T + lin_b

Distribution (v5): 8 NeuronCores = 4 q-blocks x 2 head-groups.  Core i
handles q-rows [1024*(i//2), +1024) for heads [4*(i%2), +4).  Each core
returns the PARTIAL final-linear product over its 128 local hidden dims;
the host sums core pairs and adds lin_b (no on-device collectives).  This
halves the per-core K/V projection work vs. pure q-sharding (K/V outputs
are 128 dims instead of 256) -- the projections were ~35us of PE wall.

Everything is computed in the transposed domain (scores^T with k-nodes on
partitions) so no on-device transposes are needed.

Engine budget per 4-head group (q-chunk 512 x k-chunk 128):
  - 4 scores MMs (row-tiled quadrants) -> st012 (3-bank tile) + st3.
  - exp split alternates by group parity: even kc ACT exps heads 0,1 in one
    [128,1024] ACTIVATE + DVE heads 2,3; odd kc ACT takes heads 0-2 in one
    [128,1536] ACTIVATE + DVE head 3.  Merged ACT calls amortize the
    ~352-cycle ACTIVATE startup; the alternation balances ACT/DVE load.
  - PV and Z (ones-row) matmuls accumulate IN PSUM across all 32 k-chunks.
  - V_bias is folded into lin_b on the host (attn rows sum to 1).
  - projections interleave into the group stream via a 2-bank PSUM work
    ring; input DMA split across both hwdge queues.

Measured laws (this hw):
  - PE HAM-throttles between K=8/8 (~379ns per 512-col MM) and K=4/8
    (~600ns) run-to-run; compare runs via the median scores-MM duration.
  - ACT ACTIVATE ~ (cols+352)/1.2GHz; DVE [128,512] PSUM op ~683ns (1x,
    fp32 PSUM source caps DVE modes).
  - do NOT interleave MMs of two accumulation chains at the same tile
    position (LDWEIGHTS clobber corrupts silently).
  - fp8 (e4m3) projections fail the 2e-2 gate (4-5e-2 end to end).
"""

import os
import sys
import numpy as np
import ml_dtypes

for _p in ("/opt/trn_rl_repo", "/root/.axon_site/_ro/trn_rl_repo"):
    if os.path.isdir(_p) and _p not in sys.path:
        sys.path.insert(0, _p)

N = 4096
IND = 256          # input_x dim
QKD = 1024         # concat dim for Q/K projections
H = 8              # heads
HD = 32            # head dim
HID = 256          # H * HD
HIDL = 128         # local hidden dims per core (4 heads x 32)
NCORES = 8
NQ2 = 1024         # q rows per core (4 q-blocks x 2 head-groups)
NQ = 512           # q chunk processed per phase
SCALE = 1.0 / 16.0  # 1/sqrt(HID)

# Schraudolph exp constants: one VectorE tensor_scalar writing int16
# produces the BF16 bit pattern of exp(s/16).  C=366000 minimizes max rel
# err (~3.3%).
A_EXP = float((2.0 ** 23) * np.log2(np.e) / 16.0 / 65536.0)
B_EXP = float((127.0 * 2.0 ** 23 - 366000.0) / 65536.0)

_CACHE = {}


def _build_nc():
    from contextlib import ExitStack
    import concourse.bacc as bacc
    import concourse.tile as tile
    import concourse.mybir as mybir
    from concourse.bass import ds, ts

    f32 = mybir.dt.float32
    i16 = mybir.dt.int16
    bf16 = mybir.dt.bfloat16
    Exp = mybir.ActivationFunctionType.Exp
    mult = mybir.AluOpType.mult
    add = mybir.AluOpType.add

    nc = bacc.Bacc("TRN2", target_bir_lowering=False, debug=False,
                   num_devices=NCORES)

    # ---- DRAM I/O (per-core shards prepared on host) ----
    xkT = nc.dram_tensor("xkT", [QKD, N], bf16, kind="ExternalInput")    # [x;peK]^T
    xqT = nc.dram_tensor("xqT", [QKD, NQ2], bf16, kind="ExternalInput")  # q-block
    wq = nc.dram_tensor("wq", [QKD, HIDL], bf16, kind="ExternalInput")   # local heads
    wk = nc.dram_tensor("wk", [QKD, HIDL], bf16, kind="ExternalInput")
    wv = nc.dram_tensor("wv", [IND, HIDL], bf16, kind="ExternalInput")
    lwT = nc.dram_tensor("lwT", [HIDL, HID], bf16, kind="ExternalInput")  # local rows
    bias2 = nc.dram_tensor("bias2", [128, 2], f32, kind="ExternalInput")  # Q,K bias
    out = nc.dram_tensor("out", [HID, NQ2], f32, kind="ExternalOutput")   # partial^T

    # Z-row gather constants (local heads j at partitions 32j).
    selz_np = np.zeros((128, 4), dtype=np.float32)
    for r in range(4):
        selz_np[32 * r, r] = 1.0
    selz_dram = nc.inline_tensor(np.ascontiguousarray(selz_np), name="selz_const")
    bsel_np = np.zeros((4, 128), dtype=np.float32)
    for j in range(4):
        bsel_np[j, 32 * j:32 * j + 32] = 1.0
    bsel_dram = nc.inline_tensor(bsel_np, name="bsel_const")
    ones_np = np.ones((128, 1), dtype=ml_dtypes.bfloat16)
    ones_dram = nc.inline_tensor(ones_np, name="ones_const")

    with tile.TileContext(nc) as tc, ExitStack() as ctx:
        consts = ctx.enter_context(tc.tile_pool(name="consts", bufs=1))
        big = ctx.enter_context(tc.tile_pool(name="big", bufs=1))
        ptp = ctx.enter_context(tc.tile_pool(name="ptp", bufs=3))
        stp = ctx.enter_context(tc.tile_pool(name="stp", bufs=1, space="PSUM"))

        # ---- SBUF tiles ----
        xkt = big.tile([128, 8, N], bf16, tag="xkt")        # x_K^T (8 c-chunks)
        xqt = big.tile([128, 8, NQ2], bf16, tag="xqt")      # x_Q^T block
        wqt = consts.tile([128, 8, HIDL], bf16, tag="wqt")
        wkt = consts.tile([128, 8, HIDL], bf16, tag="wkt")
        wvt = consts.tile([128, 2, HIDL], bf16, tag="wvt")
        lwt = consts.tile([128, HID], bf16, tag="lwt")      # [local-dim, out]
        bt = consts.tile([128, 2], f32, tag="bt")
        selz = consts.tile([128, 4], f32, tag="selz")
        bsel = consts.tile([4, 128], f32, tag="bsel")
        ones = consts.tile([128, 1], bf16, tag="ones")

        kt = big.tile([128, N], bf16, tag="kt")             # K^T (local heads)
        qt = big.tile([128, 2, NQ], bf16, tag="qt")         # Q^T per q-phase
        vt = big.tile([128, 32, HIDL], bf16, tag="vt")      # V node-major
        pvs = big.tile([128, 2, NQ], f32, tag="pvs")        # PV accum drains
        zsb = big.tile([128, 2, NQ], f32, tag="zsb")        # Z accum drains
        zrm = big.tile([4, 2, NQ], f32, tag="zrm")          # 1/Z per head
        attn = big.tile([128, 2, NQ], bf16, tag="attn")     # normalized attn_x^T
        outsb = big.tile([128, 2, 2, NQ], f32, tag="outsb")  # [p, mo, qb, q]

        # ---- const / weight DMAs, ordered by first consumer ----
        # group 0 gates on: wq+xqt (q-proj), wk+xkt-c0 (k-proj); wvt+ones
        # next (first pvz); bt at the first drains; lwt/selz/bsel only at
        # normalization/epilogue -> queue them last.
        xkT_r = xkT.rearrange("(c p) (n q) -> n p c q", p=128, q=512)
        xqT_r = xqT.rearrange("(c p) q -> p c q", p=128)
        nc.sync.dma_start(wqt[:], wq.rearrange("(c p) o -> p c o", p=128))
        nc.sync.dma_start(wkt[:], wk.rearrange("(c p) o -> p c o", p=128))
        # xqt split by q-phase: q-proj(0) only needs cols 0:512 (all 8 c),
        # so phase-0's half lands first; phase-1's half arrives mid-stream
        # (consumed around group 20).
        nc.scalar.dma_start(xqt[:, :, ds(0, 512)], xqT_r[:, :, ds(0, 512)])
        nc.scalar.dma_start(xkt[:, :, ds(0, 128)], xkT_r[0][:, :, ds(0, 128)])
        nc.scalar.dma_start(bt[:], bias2[:])
        nc.sync.dma_start(xkt[:, :, ds(128, 384)], xkT_r[0][:, :, ds(128, 384)])
        nc.sync.dma_start(wvt[:], wv.rearrange("(c p) o -> p c o", p=128))
        nc.sync.dma_start(ones[:], ones_dram[:])
        # dual-queue: scalar's HWDGE queue carries half of each node chunk
        # concurrently with sync's half
        for n in range(1, 6):
            nc.scalar.dma_start(xkt[:, :4, ts(n, 512)], xkT_r[n][:, :4])
            nc.sync.dma_start(xkt[:, 4:, ts(n, 512)], xkT_r[n][:, 4:])
        nc.scalar.dma_start(xqt[:, :, ds(512, 512)], xqT_r[:, :, ds(512, 512)])
        for n in range(6, 8):
            nc.scalar.dma_start(xkt[:, :4, ts(n, 512)], xkT_r[n][:, :4])
            nc.sync.dma_start(xkt[:, 4:, ts(n, 512)], xkT_r[n][:, 4:])
        nc.sync.dma_start(lwt[:], lwT[:])
        nc.sync.dma_start(selz[:], selz_dram[:])
        nc.sync.dma_start(bsel[:], bsel_dram[:])

        # preload the ACT exp table set while DMAs land
        actwarm = consts.tile([8, 16], f32, tag="actwarm")
        nc.vector.memset(actwarm[:], 0.0)
        nc.scalar.activation(actwarm[:], actwarm[:], Exp)

        # ---- projection units (PSUM work ring, 2 banks) ----
        # Two independent half-units interleave their MMs so the PE streams
        # 2-wide (adjacent MMs hit different banks).
        def qk_proj_pair(specs):
            # specs: list of 2 (kind, n) where kind in 'q','k'; for 'q' n is
            # the q-phase, for 'k' the 512-node block.
            tiles = []
            for kind, n in specs:
                ps = stp.tile([128, NQ], f32, tag="work", bufs=2,
                              name=f"{kind}p{n}")
                tiles.append(ps)
            for (kind, n), ps in zip(specs, tiles):
                for c in range(8):
                    if kind == 'q':
                        nc.tensor.matmul(ps[:, :NQ], wqt[:, c, :],
                                         xqt[:, c, ds(512 * n, 512)],
                                         start=(c == 0), stop=(c == 7))
                    else:
                        nc.tensor.matmul(ps[:, :NQ], wkt[:, c, :],
                                         xkt[:, c, ts(n, 512)],
                                         start=(c == 0), stop=(c == 7))
            for (kind, n), ps in zip(specs, tiles):
                if kind == 'q':
                    nc.vector.tensor_scalar_add(qt[:, n, :], ps[:, :NQ],
                                                bt[:, 0:1])
                else:
                    nc.vector.tensor_scalar_add(kt[:, ts(n, 512)], ps[:, :NQ],
                                                bt[:, 1:2])

        def v_proj_unit(g):
            # covers node chunks 2g, 2g+1 -> vt[:, 2g:2g+2, :] (2x128 cols).
            # Drains alternate VectorE/ScalarE to balance the exp queues.
            ps = stp.tile([128, NQ], f32, tag="work", bufs=2, name=f"vp{g}")
            for kc in (2 * g, 2 * g + 1):
                off = HIDL * (kc - 2 * g)
                for c in range(2):
                    nc.tensor.matmul(ps[:, ds(off, HIDL)],
                                     xkt[:, c, ds(128 * kc, 128)],
                                     wvt[:, c, :], start=(c == 0), stop=(c == 1))
            src = ps[:, ds(0, 2 * HIDL)].rearrange("p (g o) -> p g o", o=HIDL)
            if g % 2 == 0:
                nc.vector.tensor_copy(out=vt[:, 2 * g:2 * g + 2, :], in_=src)
            else:
                nc.scalar.copy(vt[:, 2 * g:2 * g + 2, :], src)

        # ---- prologue: minimum for attention group (kc=0, qb=0) ----
        qk_proj_pair([('q', 0), ('k', 0)])
        v_proj_unit(0)

        # remaining proj work, scheduled into the group stream.
        # group index g = 32*qb + kc (qb outer).  k_proj(n) must complete
        # before group 4n; v_proj(g') before group 2g'; q(1) before group 32.
        pre_work = {}

        def sched(slot, fn):
            pre_work.setdefault(max(0, slot), []).append(fn)

        sched(0, lambda: qk_proj_pair([('k', 1)]))
        sched(3, lambda: qk_proj_pair([('k', 2), ('k', 3)]))
        sched(9, lambda: qk_proj_pair([('k', 4), ('k', 5)]))
        sched(17, lambda: qk_proj_pair([('k', 6), ('k', 7)]))
        sched(20, lambda: qk_proj_pair([('q', 1)]))  # xqt phase-1 lands late
        for g in range(1, 16):
            sched(2 * g - 2, lambda g=g: v_proj_unit(g))

        # ---- main attention loop: qb outer, 32 k-chunks inner ----
        def pvz_unit(pt, kc, pvacc, zacc):
            for j in range(4):
                nc.tensor.matmul(
                    pvacc[ds(32 * j, 32), :],
                    vt[:, kc, ds(32 * j, 32)],
                    pt[:, ts(j, NQ)],
                    start=(kc == 0), stop=(kc == 31),
                    tile_position=(0, 32 * j))
            for j in range(4):
                nc.tensor.matmul(
                    zacc[ds(32 * j, 1), :],
                    ones[:],
                    pt[:, ts(j, NQ)],
                    start=(kc == 0), stop=(kc == 31),
                    tile_position=(0, 32 * j))

        prev = None
        # accumulators live across both q-phases (start=True reopens the
        # accumulation each phase); rows of zacc outside {0,32,64,96} are
        # never PE-written but flow into the selz gather (x0.0) -- one
        # memset keeps them finite for the whole kernel.
        pvacc = stp.tile([128, NQ], f32, tag="pv", bufs=1, name="pvacc")
        zacc = stp.tile([128, NQ], f32, tag="z", bufs=1, name="zacc")
        nc.vector.memset(zacc[:], 0.0)
        for qb in range(2):
            for kc in range(32):
                g = 32 * qb + kc
                # heads 0-2 share a 3-bank tile st012; head 3 gets st3.
                st012 = stp.tile([128, 3 * NQ], f32, tag="st012", bufs=1,
                                 name="st012")
                st3 = stp.tile([128, NQ], f32, tag="st3", bufs=1, name="st3")
                sth = [st012[:, ds(0, NQ)], st012[:, ds(NQ, NQ)],
                       st012[:, ds(2 * NQ, NQ)], st3]
                # ACT exps 3 heads on odd groups, 2 on even (~2.5 avg)
                # balancing ACT (merged-call rate) against DVE (per-op
                # overhead + drain work).
                act3 = (kc % 2 == 1)
                prev3 = (kc % 2 == 0 and kc > 0)
                # issue order: the bank freed earliest by the previous
                # group's exp goes first (prev 3/1 group: st3 freed first).
                jorder = [3, 0, 1, 2] if prev3 else [0, 1, 2, 3]
                for j in jorder:
                    nc.tensor.matmul(
                        sth[j][:, :NQ],
                        kt[ds(32 * j, 32), ds(128 * kc, 128)],
                        qt[ds(32 * j, 32), qb, :],
                        start=True, stop=True,
                        tile_position=(32 * j, 0))
                pt = ptp.tile([128, 4 * NQ], bf16, tag="pt", name="pt")
                pti = pt.bitcast(i16)
                if act3:
                    nc.scalar.activation(pt[:, ds(0, 3 * NQ)],
                                         st012[:, :], Exp, scale=SCALE)
                    nc.vector.tensor_scalar(pti[:, ds(3 * NQ, NQ)],
                                            st3[:, :NQ],
                                            A_EXP, B_EXP, mult, add)
                else:
                    nc.scalar.activation(pt[:, ds(0, 2 * NQ)],
                                         st012[:, ds(0, 2 * NQ)], Exp,
                                         scale=SCALE)
                    nc.vector.tensor_scalar(pti[:, ds(2 * NQ, NQ)],
                                            st012[:, ds(2 * NQ, NQ)],
                                            A_EXP, B_EXP, mult, add)
                    nc.vector.tensor_scalar(pti[:, ds(3 * NQ, NQ)],
                                            st3[:, :NQ],
                                            A_EXP, B_EXP, mult, add)
                # pvz of the previous group goes BEFORE this slot's proj
                # units: the PE is in-order, and the projections are filler
                # that must not delay the accumulation stream.
                if prev is not None:
                    pvz_unit(*prev)
                for fn in pre_work.get(g, []):
                    fn()
                prev = (pt, kc, pvacc, zacc)
            pvz_unit(*prev)
            prev = None
            # drain this qb's accumulators; normalization chain (qb0's hides
            # inside the qb1 phase; only qb1's is an exposed tail).
            nc.vector.tensor_copy(out=pvs[:, qb, :], in_=pvacc[:])
            # zsb drain on ScalarE: runs concurrently with the pvs copy.
            nc.scalar.copy(zsb[:, qb, :], zacc[:])
            zqm = stp.tile([128, NQ], f32, tag="work", bufs=2, name=f"zq{qb}")
            nc.tensor.matmul(zqm[:4, :NQ], selz[:], zsb[:, qb, :],
                             start=True, stop=True)
            nc.vector.reciprocal_approx_fast(zrm[:, qb, :], zqm[:4, :NQ])
            psb = stp.tile([128, NQ], f32, tag="work", bufs=2, name=f"psb{qb}")
            nc.tensor.matmul(psb[:, :NQ], bsel[:], zrm[:4, qb, :],
                             start=True, stop=True)
            # V_bias is folded into lin_b on the host (attn rows sum to 1).
            nc.vector.tensor_tensor(attn[:, qb, :], pvs[:, qb, :], psb[:, :NQ],
                                    mult)
            # partial final linear for this q-phase (contract = 128 local
            # dims, single chunk); qb0's hides inside the qb1 phase.
            out_r = out.rearrange("(m p) q -> p m q", p=128)
            for mo in range(2):
                lin = stp.tile([128, NQ], f32, tag="work", bufs=2,
                               name=f"lin{qb}_{mo}")
                nc.tensor.matmul(lin[:, :NQ], lwt[:, ts(mo, 128)],
                                 attn[:, qb, :], start=True, stop=True)
                nc.scalar.copy(outsb[:, mo, qb, :], lin[:, :NQ])
                nc.sync.dma_start(out_r[:, mo, ds(512 * qb, 512)],
                                  outsb[:, mo, qb, :])

    nc.compile()
    return nc


def _get_nc():
    if "nc" not in _CACHE:
        _CACHE["nc"] = _build_nc()
    return _CACHE["nc"]


def _prep_in_maps(input_x, pe_Q, pe_K, WQ, WK, WV, Q_bias, K_bias, V_bias,
                  lin_w, lin_b):
    bf = ml_dtypes.bfloat16
    x_kT = np.ascontiguousarray(
        np.concatenate([input_x, pe_K], axis=1).T.astype(bf))       # [1024, 4096]
    x_q = np.concatenate([input_x, pe_Q], axis=1)                   # [4096, 1024]
    wq_h = WQ.transpose(1, 0, 2).reshape(QKD, HID)                  # [d,(h,hd)]
    wk_h = WK.transpose(1, 0, 2).reshape(QKD, HID)
    wv_h = WV.transpose(1, 0, 2).reshape(IND, HID)
    lwTn = lin_w.T                                                  # [in, out]
    qb_full = Q_bias.reshape(HID)
    kb_full = K_bias.reshape(HID)
    in_maps = []
    for i in range(NCORES):
        blk, hg = i // 2, i % 2
        xqT_i = np.ascontiguousarray(
            x_q[blk * NQ2:(blk + 1) * NQ2].T.astype(bf))            # [1024, 1024]
        sl = slice(HIDL * hg, HIDL * (hg + 1))
        bias2 = np.zeros((128, 2), np.float32)
        bias2[:, 0] = qb_full[sl]
        bias2[:, 1] = kb_full[sl]
        in_maps.append({
            "xkT": x_kT, "xqT": xqT_i,
            "wq": np.ascontiguousarray(wq_h[:, sl].astype(bf)),
            "wk": np.ascontiguousarray(wk_h[:, sl].astype(bf)),
            "wv": np.ascontiguousarray(wv_h[:, sl].astype(bf)),
            "lwT": np.ascontiguousarray(lwTn[sl, :].astype(bf)),
            "bias2": bias2,
        })
    return in_maps


def _ensure_ntff_hook():
    """The agent image's antenv lacks axon_hooks; synthesize it from the
    boot script's ctypes NTFF implementation so trace=True works."""
    import types
    try:
        from antenv.axon_hooks import get_axon_ntff_profile_hook  # noqa: F401
        return
    except ImportError:
        pass
    sys.path.insert(0, "/root/.axon_site/trn_agent_boot")
    import trn_boot
    hook = trn_boot._ntff_profile_via_ctypes(
        os.environ.get("PJRT_LIBRARY_PATH", "/opt/axon/libaxon_pjrt.so"))
    mod = types.ModuleType("antenv.axon_hooks")
    mod._hook = hook
    mod.get_axon_ntff_profile_hook = lambda: mod._hook
    mod.set_axon_ntff_profile_hook = lambda h: setattr(mod, "_hook", h)
    sys.modules["antenv.axon_hooks"] = mod


def _run(in_maps, trace=False):
    from concourse.bass_utils import run_bass_kernel_spmd
    if trace:
        _ensure_ntff_hook()
    nc = _get_nc()
    res = run_bass_kernel_spmd(nc, in_maps, core_ids=list(range(NCORES)),
                               trace=trace)
    return res


def _finish(res, V_bias, lin_w, lin_b):
    # host epilogue: sum the two head-group partials per q-block; V_bias
    # (constant through the softmax) and lin_b fold into one bias vector.
    lin_b_eff = (lin_b.reshape(HID) +
                 V_bias.reshape(HID) @ lin_w.T).astype(np.float32)
    out_full = np.empty((N, HID), np.float32)
    for blk in range(4):
        part = res.results[2 * blk]["out"] + res.results[2 * blk + 1]["out"]
        out_full[blk * NQ2:(blk + 1) * NQ2] = part.T + lin_b_eff
    return out_full


def kernel(input_x, pe_Q, pe_K, A, WQ, WK, WV, Q_bias, K_bias, V_bias,
           lin_w, lin_b):
    args = [np.asarray(x, np.float32) for x in
            (input_x, pe_Q, pe_K, WQ, WK, WV, Q_bias, K_bias, V_bias,
             lin_w, lin_b)]
    in_maps = _prep_in_maps(*args)
    res = _run(in_maps)
    return _finish(res, args[8], args[9], args[10])


def hw_exec_ns(input_x, pe_Q, pe_K, A, WQ, WK, WV, Q_bias, K_bias, V_bias,
               lin_w, lin_b):
    """Run once with NTFF tracing; returns (exec_time_ns, results)."""
    args = [np.asarray(x, np.float32) for x in
            (input_x, pe_Q, pe_K, WQ, WK, WV, Q_bias, K_bias, V_bias,
             lin_w, lin_b)]
    in_maps = _prep_in_maps(*args)
    res = _run(in_maps, trace=True)
    return res.exec_time_ns, res
